# revision 2
# baseline (speedup 1.0000x reference)
"""AssemblyClassifier Trainium2 kernel v3: 8-way batch-parallel across cores.

v1 structure (contiguous 16KB/partition chunk loads, 128-partition write
DMAs) with three changes:
 - read DMA descriptors forced to 4KB by padding the SBUF destination rows
   (dst stride 4160B breaks the contiguous run): ~236ns/4KB vs ~1089ns/16KB
   per descriptor -> read stream ~52us instead of ~61us of engine time.
 - all output DMAs + table loads ride the scalar (ACT) HW queue; reads ride
   the sync (SP) queue front-loaded, so writes drain during the read phase.
 - PSUM evacuated with one 4-bank copy per jp (2 per chunk instead of 8).
"""
import os
import sys
import types

import numpy as np

_B, _E, _T, _F = 8, 28, 16384, 8
_A, _C = 1024, 256
_HI = 4                 # t_hi groups (partition dim = 4*e + t_hi)
_TL = _T // _HI         # 4096 t_lo per group
_NT = 512               # t_lo chunk
_NCHUNK = _TL // _NT    # 8
_P = _E * _HI           # 112 active partitions
_SUB = 4                # 4KB sub-blocks per chunk row
_SW = 1024              # f32 per sub-block (4KB)
_PAD = 16               # f32 padding per sub-block row in SBUF

_cache = {}
LAST_RESULTS = None


def _ensure_axon_hooks():
    try:
        import antenv.axon_hooks  # noqa: F401
        return
    except Exception:
        pass
    try:
        from trn_agent_boot.trn_boot import _ntff_profile_via_ctypes
        hook = _ntff_profile_via_ctypes('/opt/axon/libaxon_pjrt.so')
    except Exception:
        hook = None
    m = types.ModuleType('antenv.axon_hooks')
    m.get_axon_ntff_profile_hook = lambda: hook
    m.set_axon_ntff_profile_hook = lambda h: None
    sys.modules['antenv.axon_hooks'] = m


def _build():
    import concourse.bass as bass
    import concourse.mybir as mybir
    from concourse import bacc
    from concourse.tile import TileContext

    F32 = mybir.dt.float32
    BF16 = mybir.dt.bfloat16
    ALU = mybir.AluOpType
    ACTF = mybir.ActivationFunctionType

    nc = bacc.Bacc("TRN2", target_bir_lowering=False)
    x = nc.declare_dram_parameter("x", [_E, _T, _F], F32, isOutput=False)
    tab = nc.declare_dram_parameter("tab", [_A, 512], BF16, isOutput=False)
    maskm = nc.declare_dram_parameter("maskm", [128, _HI], F32, isOutput=False)
    out = nc.declare_dram_parameter("out", [_T, _C], BF16, isOutput=True)

    # flat [112, (tl f)] view of x; partition p = 4*e + t_hi
    xv = x[:].rearrange("e (hi tl) f -> (e hi) (tl f)", hi=_HI)

    with TileContext(nc) as tc:
        with (
            tc.tile_pool(name="const", bufs=1) as constp,
            tc.tile_pool(name="xin", bufs=6) as xin,
            tc.tile_pool(name="work", bufs=2) as work,
            tc.tile_pool(name="mm", bufs=3) as mmp,
            tc.tile_pool(name="psum", bufs=2, space="PSUM") as psp,
            tc.tile_pool(name="outp", bufs=4) as outp,
        ):
            # ---------------- G2 table prep (ACT queue) ----------------
            tab_sb = constp.tile([128, _A // 128, 512], BF16)
            nc.scalar.dma_start(out=tab_sb[:],
                                in_=tab[:].rearrange("(c p) n -> p c n", p=128))
            eq_sb = tab_sb[:, :, 0:256].rearrange("p c n -> p c n")
            efp_sb = tab_sb[:, :, 256:512].rearrange("p c n -> p c n")
            mask_sb = constp.tile([128, _HI], F32)
            nc.scalar.dma_start(out=mask_sb[:], in_=maskm[:])

            def load_chunk(ci, xt):
                src = xv[:, ci * _NT * _F:(ci + 1) * _NT * _F].rearrange(
                    "p (s d) -> p s d", d=_SW)
                nc.sync.dma_start(out=xt[0:_P, :, 0:_SW], in_=src)

            # prefetch chunk 0 right away on the sync (read) queue
            xt0 = xin.tile([128, _SUB, _SW + _PAD], F32, name="xt")
            load_chunk(0, xt0)

            g2m = [constp.tile([128, 2 * _C], BF16, name=f"g2m{g}")
                   for g in range(_HI)]
            for k in range(2):
                pp = psp.tile([128, 2048], mybir.dt.float32, name="pp")
                for ci in range(_A // 128):
                    nc.tensor.matmul(pp[0:_P, 0:_C],
                                     efp_sb[:, ci, 128 * k:128 * k + _P],
                                     eq_sb[:, ci, :],
                                     start=(ci == 0), stop=(ci == _A // 128 - 1))
                for g in range(_HI):
                    nc.scalar.activation(out=g2m[g][0:_P, k * _C:(k + 1) * _C],
                                         in_=pp[0:_P, 0:_C], func=ACTF.Copy,
                                         scale=mask_sb[0:_P, g:g + 1])

            # ---------------- main pipeline ----------------
            # full chunks: t = g*4096 + q*512 + p*4 + j
            ov = out[:].rearrange("(g q p j) c -> q p g (j c)", g=_HI,
                                  q=_NCHUNK, p=128, j=4)
            # half chunks: t = g*4096 + w*256 + p*2 + j
            ovw = out[:].rearrange("(g w p j) c -> w p g (j c)", g=_HI,
                                   w=_TL // 256, p=128, j=2)

            def fsum(xt):
                """DVE f-sum tree + NaN select for one full chunk."""
                nt = _NT
                xt4 = xt[0:_P, :, 0:_SW].rearrange("p s (tl f) -> p s tl f", f=8)
                l1 = work.tile([128, nt * 4], F32, name="l1")
                l1v = l1[0:_P, :].rearrange("p (s tl f) -> p s tl f", s=_SUB, f=4)
                nc.vector.tensor_tensor(out=l1v, in0=xt4[:, :, :, 0:4],
                                        in1=xt4[:, :, :, 4:8], op=ALU.add)
                l13 = l1[0:_P, :].rearrange("p (tl f) -> p tl f", f=4)
                l2 = work.tile([128, nt * 2], F32, name="l2")
                nc.vector.tensor_tensor(out=l2[0:_P, :], in0=l13[:, :, 0:2],
                                        in1=l13[:, :, 2:4], op=ALU.add)
                l23 = l2[0:_P, :].rearrange("p (tl f) -> p tl f", f=2)
                s_raw = work.tile([128, nt], F32, name="s_raw")
                nc.vector.tensor_tensor(out=s_raw[0:_P, :], in0=l23[:, :, 0:1],
                                        in1=l23[:, :, 1:2], op=ALU.add)
                obsf = mmp.tile([128, nt], BF16, name="obsf")
                nc.vector.tensor_tensor(out=obsf[0:_P, :], in0=s_raw[0:_P, :],
                                        in1=s_raw[0:_P, :], op=ALU.is_equal)
                s0 = mmp.tile([128, nt], BF16, name="s0")
                nc.vector.scalar_tensor_tensor(out=s0[0:_P, :],
                                               in0=s_raw[0:_P, :],
                                               scalar=3.0e38, in1=obsf[0:_P, :],
                                               op0=ALU.min, op1=ALU.mult)
                return s0, obsf

            def mm_full(s0, obsf, wslot):
                """matmuls + evac + out-DMA for one full 512-col chunk."""
                og = outp.tile([128, _HI * 4 * _C], BF16, name="og")
                # og columns: g*1024 + j*256 + c  with j = 2*jp + jj
                ogv = og[:, :].rearrange("p (g jp jj c) -> p g jp jj c",
                                         g=_HI, jp=2, jj=2)
                for jp in range(2):
                    pp = psp.tile([128, 2048], mybir.dt.float32, name="pp")
                    for jj in range(2):
                        j = 2 * jp + jj
                        lhs_s0 = s0[0:_P, j::4]
                        lhs_ob = obsf[0:_P, j::4]
                        for g in range(_HI):
                            sl = pp[:, (g * 2 + jj) * _C:(g * 2 + jj + 1) * _C]
                            nc.tensor.matmul(sl, lhs_s0, g2m[g][0:_P, 0:_C],
                                             start=True, stop=False)
                            nc.tensor.matmul(sl, lhs_ob, g2m[g][0:_P, _C:2 * _C],
                                             start=False, stop=True)
                    nc.scalar.copy(out=ogv[:, :, jp, :, :],
                                   in_=pp[:].rearrange("p (g jj c) -> p g jj c",
                                                       g=_HI, jj=2))
                nc.scalar.dma_start(out=ov[wslot],
                                    in_=og[:].rearrange("p (g jc) -> p g jc",
                                                        g=_HI))

            def mm_half(s0, obsf, h, wslot):
                """matmuls + evac + out-DMA for one 256-col half chunk."""
                og = outp.tile([128, _HI * 2 * _C], BF16, name="ogh")
                pp = psp.tile([128, 2048], mybir.dt.float32, name="pp")
                for j in range(2):
                    lhs_s0 = s0[0:_P, h * 256 + j:(h + 1) * 256:2]
                    lhs_ob = obsf[0:_P, h * 256 + j:(h + 1) * 256:2]
                    for g in range(_HI):
                        sl = pp[:, (g * 2 + j) * _C:(g * 2 + j + 1) * _C]
                        nc.tensor.matmul(sl, lhs_s0, g2m[g][0:_P, 0:_C],
                                         start=True, stop=False)
                        nc.tensor.matmul(sl, lhs_ob, g2m[g][0:_P, _C:2 * _C],
                                         start=False, stop=True)
                nc.scalar.copy(out=og[:, :], in_=pp[:])
                nc.scalar.dma_start(out=ovw[wslot],
                                    in_=og[:].rearrange("p (g jc) -> p g jc",
                                                        g=_HI))

            for ci in range(_NCHUNK):
                if ci == 0:
                    xt = xt0
                else:
                    xt = xin.tile([128, _SUB, _SW + _PAD], F32, name="xt")
                    with tc.high_priority():
                        load_chunk(ci, xt)
                s0, obsf = fsum(xt)
                if ci < _NCHUNK - 1:
                    mm_full(s0, obsf, ci)
                else:
                    # split the final chunk to shorten the pipeline drain
                    mm_half(s0, obsf, 0, 2 * ci)
                    mm_half(s0, obsf, 1, 2 * ci + 1)
    nc.compile()
    return nc


def _get_nc():
    if "nc" not in _cache:
        _ensure_axon_hooks()
        from concourse import bass_utils
        bass_utils.upload_artifacts = lambda tmpdir: "local://skipped"
        _cache["nc"] = _build()
    return _cache["nc"]


def kernel(input_seq, eq_classes, scale, alpha, edge_present):
    global LAST_RESULTS
    x = np.asarray(input_seq, dtype=np.float32)
    eqc = np.asarray(eq_classes, dtype=np.float32)
    ef = np.asarray(edge_present).astype(np.float32)
    sc = float(np.asarray(scale))
    al = float(np.asarray(alpha))

    import ml_dtypes
    efp = np.zeros((_A, 256), np.float32)
    efp[:, 0:_P] = np.repeat(-sc * ef, _HI, axis=1)
    efp[:, 128:128 + _P] = np.repeat(al * (1.0 - ef), _HI, axis=1)
    tab = np.concatenate([eqc, efp], axis=1).astype(ml_dtypes.bfloat16)
    maskm = np.zeros((128, _HI), np.float32)
    for g in range(_HI):
        maskm[g::_HI, g] = 1.0
    maskm[_P:, :] = 0.0

    nc = _get_nc()
    from concourse import bass_utils
    in_maps = [{"x": np.ascontiguousarray(x[b]), "tab": tab,
                "maskm": maskm} for b in range(_B)]
    trace = bool(os.environ.get("KERNEL_TRACE"))
    res = bass_utils.run_bass_kernel_spmd(nc, in_maps, core_ids=list(range(_B)),
                                          trace=trace)
    LAST_RESULTS = res
    return np.stack([np.asarray(res.results[b]["out"]).astype(np.float32) for b in range(_B)], axis=0)


# revision 3
# speedup vs baseline: 1.0078x; 1.0078x over previous
"""AssemblyClassifier Trainium2 kernel v3: 8-way batch-parallel across cores.

v1 structure (contiguous 16KB/partition chunk loads, 128-partition write
DMAs) with three changes:
 - read DMA descriptors forced to 4KB by padding the SBUF destination rows
   (dst stride 4160B breaks the contiguous run): ~236ns/4KB vs ~1089ns/16KB
   per descriptor -> read stream ~52us instead of ~61us of engine time.
 - all output DMAs + table loads ride the scalar (ACT) HW queue; reads ride
   the sync (SP) queue front-loaded, so writes drain during the read phase.
 - PSUM evacuated with one 4-bank copy per jp (2 per chunk instead of 8).
"""
import os
import sys
import types

import numpy as np

_B, _E, _T, _F = 8, 28, 16384, 8
_A, _C = 1024, 256
_HI = 4                 # t_hi groups (partition dim = 4*e + t_hi)
_TL = _T // _HI         # 4096 t_lo per group
_NT = 512               # t_lo chunk
_NCHUNK = _TL // _NT    # 8
_P = _E * _HI           # 112 active partitions
_SUB = 4                # 4KB sub-blocks per chunk row
_SW = 1024              # f32 per sub-block (4KB)
_PAD = 16               # f32 padding per sub-block row in SBUF

_cache = {}
LAST_RESULTS = None


def _ensure_axon_hooks():
    try:
        import antenv.axon_hooks  # noqa: F401
        return
    except Exception:
        pass
    try:
        from trn_agent_boot.trn_boot import _ntff_profile_via_ctypes
        hook = _ntff_profile_via_ctypes('/opt/axon/libaxon_pjrt.so')
    except Exception:
        hook = None
    m = types.ModuleType('antenv.axon_hooks')
    m.get_axon_ntff_profile_hook = lambda: hook
    m.set_axon_ntff_profile_hook = lambda h: None
    sys.modules['antenv.axon_hooks'] = m


def _build():
    import concourse.bass as bass
    import concourse.mybir as mybir
    from concourse import bacc
    from concourse.tile import TileContext

    F32 = mybir.dt.float32
    BF16 = mybir.dt.bfloat16
    ALU = mybir.AluOpType
    ACTF = mybir.ActivationFunctionType

    nc = bacc.Bacc("TRN2", target_bir_lowering=False)
    x = nc.declare_dram_parameter("x", [_E, _T, _F], F32, isOutput=False)
    tab = nc.declare_dram_parameter("tab", [_A, 512], BF16, isOutput=False)
    maskm = nc.declare_dram_parameter("maskm", [128, _HI], F32, isOutput=False)
    out = nc.declare_dram_parameter("out", [_T, _C], BF16, isOutput=True)

    # flat [112, (tl f)] view of x; partition p = 4*e + t_hi
    xv = x[:].rearrange("e (hi tl) f -> (e hi) (tl f)", hi=_HI)

    with TileContext(nc) as tc:
        with (
            tc.tile_pool(name="const", bufs=1) as constp,
            tc.tile_pool(name="xin", bufs=6) as xin,
            tc.tile_pool(name="work", bufs=2) as work,
            tc.tile_pool(name="mm", bufs=3) as mmp,
            tc.tile_pool(name="psum", bufs=2, space="PSUM") as psp,
            tc.tile_pool(name="outp", bufs=4) as outp,
        ):
            # ---------------- G2 table prep (ACT queue) ----------------
            tab_sb = constp.tile([128, _A // 128, 512], BF16)
            nc.scalar.dma_start(out=tab_sb[:],
                                in_=tab[:].rearrange("(c p) n -> p c n", p=128))
            eq_sb = tab_sb[:, :, 0:256].rearrange("p c n -> p c n")
            efp_sb = tab_sb[:, :, 256:512].rearrange("p c n -> p c n")
            mask_sb = constp.tile([128, _HI], F32)
            nc.scalar.dma_start(out=mask_sb[:], in_=maskm[:])

            def load_chunk(ci, xt):
                src = xv[:, ci * _NT * _F:(ci + 1) * _NT * _F].rearrange(
                    "p (s d) -> p s d", d=_SW)
                nc.sync.dma_start(out=xt[0:_P, :, 0:_SW], in_=src)

            # prefetch chunk 0 right away on the sync (read) queue
            xt0 = xin.tile([128, _SUB, _SW + _PAD], F32, name="xt")
            load_chunk(0, xt0)

            g2m = [constp.tile([128, 2 * _C], BF16, name=f"g2m{g}")
                   for g in range(_HI)]
            for k in range(2):
                pp = psp.tile([128, 2048], mybir.dt.float32, name="pp")
                for ci in range(_A // 128):
                    nc.tensor.matmul(pp[0:_P, 0:_C],
                                     efp_sb[:, ci, 128 * k:128 * k + _P],
                                     eq_sb[:, ci, :],
                                     start=(ci == 0), stop=(ci == _A // 128 - 1))
                for g in range(_HI):
                    nc.scalar.activation(out=g2m[g][0:_P, k * _C:(k + 1) * _C],
                                         in_=pp[0:_P, 0:_C], func=ACTF.Copy,
                                         scale=mask_sb[0:_P, g:g + 1])

            # ---------------- main pipeline ----------------
            # full chunks: t = g*4096 + q*512 + p*4 + j
            ov = out[:].rearrange("(g q p j) c -> q p g (j c)", g=_HI,
                                  q=_NCHUNK, p=128, j=4)
            # half chunks: t = g*4096 + w*256 + p*2 + j
            ovw = out[:].rearrange("(g w p j) c -> w p g (j c)", g=_HI,
                                   w=_TL // 256, p=128, j=2)

            def fsum(xt):
                """DVE f-sum tree + NaN select for one full chunk."""
                nt = _NT
                xt4 = xt[0:_P, :, 0:_SW].rearrange("p s (tl f) -> p s tl f", f=8)
                l1 = work.tile([128, nt * 4], F32, name="l1")
                l1v = l1[0:_P, :].rearrange("p (s tl f) -> p s tl f", s=_SUB, f=4)
                nc.vector.tensor_tensor(out=l1v, in0=xt4[:, :, :, 0:4],
                                        in1=xt4[:, :, :, 4:8], op=ALU.add)
                l13 = l1[0:_P, :].rearrange("p (tl f) -> p tl f", f=4)
                l2 = work.tile([128, nt * 2], F32, name="l2")
                nc.vector.tensor_tensor(out=l2[0:_P, :], in0=l13[:, :, 0:2],
                                        in1=l13[:, :, 2:4], op=ALU.add)
                l23 = l2[0:_P, :].rearrange("p (tl f) -> p tl f", f=2)
                s_raw = work.tile([128, nt], F32, name="s_raw")
                nc.vector.tensor_tensor(out=s_raw[0:_P, :], in0=l23[:, :, 0:1],
                                        in1=l23[:, :, 1:2], op=ALU.add)
                obsf = mmp.tile([128, nt], BF16, name="obsf")
                nc.vector.tensor_tensor(out=obsf[0:_P, :], in0=s_raw[0:_P, :],
                                        in1=s_raw[0:_P, :], op=ALU.is_equal)
                s0 = mmp.tile([128, nt], BF16, name="s0")
                nc.vector.scalar_tensor_tensor(out=s0[0:_P, :],
                                               in0=s_raw[0:_P, :],
                                               scalar=3.0e38, in1=obsf[0:_P, :],
                                               op0=ALU.min, op1=ALU.mult)
                return s0, obsf

            def mm_full(s0, obsf, wslot):
                """matmuls + evac + out-DMA for one full 512-col chunk."""
                og = outp.tile([128, _HI * 4 * _C], BF16, name="og")
                # og columns: g*1024 + j*256 + c  with j = 2*jp + jj
                ogv = og[:, :].rearrange("p (g jp jj c) -> p g jp jj c",
                                         g=_HI, jp=2, jj=2)
                for jp in range(2):
                    pp = psp.tile([128, 2048], mybir.dt.float32, name="pp")
                    for jj in range(2):
                        j = 2 * jp + jj
                        lhs_s0 = s0[0:_P, j::4]
                        lhs_ob = obsf[0:_P, j::4]
                        for g in range(_HI):
                            sl = pp[:, (g * 2 + jj) * _C:(g * 2 + jj + 1) * _C]
                            nc.tensor.matmul(sl, lhs_s0, g2m[g][0:_P, 0:_C],
                                             start=True, stop=False)
                            nc.tensor.matmul(sl, lhs_ob, g2m[g][0:_P, _C:2 * _C],
                                             start=False, stop=True)
                    nc.scalar.copy(out=ogv[:, :, jp, :, :],
                                   in_=pp[:].rearrange("p (g jj c) -> p g jj c",
                                                       g=_HI, jj=2))
                nc.scalar.dma_start(out=ov[wslot],
                                    in_=og[:].rearrange("p (g jc) -> p g jc",
                                                        g=_HI))

            def mm_half(s0, obsf, h, wslot):
                """matmuls + evac + out-DMA for one 256-col half chunk."""
                og = outp.tile([128, _HI * 2 * _C], BF16, name="ogh")
                pp = psp.tile([128, 2048], mybir.dt.float32, name="pp")
                for j in range(2):
                    lhs_s0 = s0[0:_P, h * 256 + j:(h + 1) * 256:2]
                    lhs_ob = obsf[0:_P, h * 256 + j:(h + 1) * 256:2]
                    for g in range(_HI):
                        sl = pp[:, (g * 2 + j) * _C:(g * 2 + j + 1) * _C]
                        nc.tensor.matmul(sl, lhs_s0, g2m[g][0:_P, 0:_C],
                                         start=True, stop=False)
                        nc.tensor.matmul(sl, lhs_ob, g2m[g][0:_P, _C:2 * _C],
                                         start=False, stop=True)
                nc.scalar.copy(out=og[:, :], in_=pp[:])
                nc.scalar.dma_start(out=ovw[wslot],
                                    in_=og[:].rearrange("p (g jc) -> p g jc",
                                                        g=_HI))

            for ci in range(_NCHUNK):
                if ci == 0:
                    xt = xt0
                else:
                    xt = xin.tile([128, _SUB, _SW + _PAD], F32, name="xt")
                    with tc.high_priority():
                        load_chunk(ci, xt)
                s0, obsf = fsum(xt)
                mm_full(s0, obsf, ci)
    nc.compile()
    return nc


def _get_nc():
    if "nc" not in _cache:
        _ensure_axon_hooks()
        from concourse import bass_utils
        bass_utils.upload_artifacts = lambda tmpdir: "local://skipped"
        _cache["nc"] = _build()
    return _cache["nc"]


def kernel(input_seq, eq_classes, scale, alpha, edge_present):
    global LAST_RESULTS
    x = np.asarray(input_seq, dtype=np.float32)
    eqc = np.asarray(eq_classes, dtype=np.float32)
    ef = np.asarray(edge_present).astype(np.float32)
    sc = float(np.asarray(scale))
    al = float(np.asarray(alpha))

    import ml_dtypes
    efp = np.zeros((_A, 256), np.float32)
    efp[:, 0:_P] = np.repeat(-sc * ef, _HI, axis=1)
    efp[:, 128:128 + _P] = np.repeat(al * (1.0 - ef), _HI, axis=1)
    tab = np.concatenate([eqc, efp], axis=1).astype(ml_dtypes.bfloat16)
    maskm = np.zeros((128, _HI), np.float32)
    for g in range(_HI):
        maskm[g::_HI, g] = 1.0
    maskm[_P:, :] = 0.0

    nc = _get_nc()
    from concourse import bass_utils
    in_maps = [{"x": np.ascontiguousarray(x[b]), "tab": tab,
                "maskm": maskm} for b in range(_B)]
    trace = bool(os.environ.get("KERNEL_TRACE"))
    res = bass_utils.run_bass_kernel_spmd(nc, in_maps, core_ids=list(range(_B)),
                                          trace=trace)
    LAST_RESULTS = res
    return np.stack([np.asarray(res.results[b]["out"]).astype(np.float32) for b in range(_B)], axis=0)


# revision 4
# speedup vs baseline: 1.0170x; 1.0092x over previous
"""AssemblyClassifier Trainium2 kernel v3: 8-way batch-parallel across cores.

v1 structure (contiguous 16KB/partition chunk loads, 128-partition write
DMAs) with three changes:
 - read DMA descriptors forced to 4KB by padding the SBUF destination rows
   (dst stride 4160B breaks the contiguous run): ~236ns/4KB vs ~1089ns/16KB
   per descriptor -> read stream ~52us instead of ~61us of engine time.
 - all output DMAs + table loads ride the scalar (ACT) HW queue; reads ride
   the sync (SP) queue front-loaded, so writes drain during the read phase.
 - PSUM evacuated with one 4-bank copy per jp (2 per chunk instead of 8).
"""
import os
import sys
import types

import numpy as np

_B, _E, _T, _F = 8, 28, 16384, 8
_A, _C = 1024, 256
_HI = 4                 # t_hi groups (partition dim = 4*e + t_hi)
_TL = _T // _HI         # 4096 t_lo per group
_NT = 512               # t_lo chunk
_NCHUNK = _TL // _NT    # 8
_P = _E * _HI           # 112 active partitions
_SUB = 4                # 4KB sub-blocks per chunk row
_SW = 1024              # f32 per sub-block (4KB)
_PAD = 16               # f32 padding per sub-block row in SBUF

_cache = {}
LAST_RESULTS = None


def _ensure_axon_hooks():
    try:
        import antenv.axon_hooks  # noqa: F401
        return
    except Exception:
        pass
    try:
        from trn_agent_boot.trn_boot import _ntff_profile_via_ctypes
        hook = _ntff_profile_via_ctypes('/opt/axon/libaxon_pjrt.so')
    except Exception:
        hook = None
    m = types.ModuleType('antenv.axon_hooks')
    m.get_axon_ntff_profile_hook = lambda: hook
    m.set_axon_ntff_profile_hook = lambda h: None
    sys.modules['antenv.axon_hooks'] = m


def _build():
    import concourse.bass as bass
    import concourse.mybir as mybir
    from concourse import bacc
    from concourse.tile import TileContext

    F32 = mybir.dt.float32
    BF16 = mybir.dt.bfloat16
    ALU = mybir.AluOpType
    ACTF = mybir.ActivationFunctionType

    nc = bacc.Bacc("TRN2", target_bir_lowering=False)
    x = nc.declare_dram_parameter("x", [_E, _T, _F], F32, isOutput=False)
    tab = nc.declare_dram_parameter("tab", [_A, 512], BF16, isOutput=False)
    maskm = nc.declare_dram_parameter("maskm", [128, _HI], F32, isOutput=False)
    out = nc.declare_dram_parameter("out", [_T, _C], BF16, isOutput=True)

    # flat [112, (tl f)] view of x; partition p = 4*e + t_hi
    xv = x[:].rearrange("e (hi tl) f -> (e hi) (tl f)", hi=_HI)

    with TileContext(nc) as tc:
        with (
            tc.tile_pool(name="const", bufs=1) as constp,
            tc.tile_pool(name="xin", bufs=6) as xin,
            tc.tile_pool(name="work", bufs=2) as work,
            tc.tile_pool(name="mm", bufs=4) as mmp,
            tc.tile_pool(name="psum", bufs=2, space="PSUM") as psp,
            tc.tile_pool(name="outp", bufs=6) as outp,
        ):
            # ---------------- G2 table prep (ACT queue) ----------------
            tab_sb = constp.tile([128, _A // 128, 512], BF16)
            nc.scalar.dma_start(out=tab_sb[:],
                                in_=tab[:].rearrange("(c p) n -> p c n", p=128))
            eq_sb = tab_sb[:, :, 0:256].rearrange("p c n -> p c n")
            efp_sb = tab_sb[:, :, 256:512].rearrange("p c n -> p c n")
            mask_sb = constp.tile([128, _HI], F32)
            nc.scalar.dma_start(out=mask_sb[:], in_=maskm[:])

            def load_chunk(ci, xt):
                src = xv[:, ci * _NT * _F:(ci + 1) * _NT * _F].rearrange(
                    "p (s d) -> p s d", d=_SW)
                nc.sync.dma_start(out=xt[0:_P, :, 0:_SW], in_=src)

            # prefetch chunk 0 right away on the sync (read) queue
            xt0 = xin.tile([128, _SUB, _SW + _PAD], F32, name="xt")
            load_chunk(0, xt0)

            g2m = [constp.tile([128, 2 * _C], BF16, name=f"g2m{g}")
                   for g in range(_HI)]
            for k in range(2):
                pp = psp.tile([128, 2048], mybir.dt.float32, name="pp")
                for ci in range(_A // 128):
                    nc.tensor.matmul(pp[0:_P, 0:_C],
                                     efp_sb[:, ci, 128 * k:128 * k + _P],
                                     eq_sb[:, ci, :],
                                     start=(ci == 0), stop=(ci == _A // 128 - 1))
                for g in range(_HI):
                    nc.scalar.activation(out=g2m[g][0:_P, k * _C:(k + 1) * _C],
                                         in_=pp[0:_P, 0:_C], func=ACTF.Copy,
                                         scale=mask_sb[0:_P, g:g + 1])

            # ---------------- main pipeline ----------------
            # full chunks: t = g*4096 + q*512 + p*4 + j
            ov = out[:].rearrange("(g q p j) c -> q p g (j c)", g=_HI,
                                  q=_NCHUNK, p=128, j=4)
            # half chunks: t = g*4096 + w*256 + p*2 + j
            ovw = out[:].rearrange("(g w p j) c -> w p g (j c)", g=_HI,
                                   w=_TL // 256, p=128, j=2)

            def fsum(xt):
                """DVE f-sum tree + NaN select for one full chunk."""
                nt = _NT
                xt4 = xt[0:_P, :, 0:_SW].rearrange("p s (tl f) -> p s tl f", f=8)
                l1 = work.tile([128, nt * 4], F32, name="l1")
                l1v = l1[0:_P, :].rearrange("p (s tl f) -> p s tl f", s=_SUB, f=4)
                nc.vector.tensor_tensor(out=l1v, in0=xt4[:, :, :, 0:4],
                                        in1=xt4[:, :, :, 4:8], op=ALU.add)
                l13 = l1[0:_P, :].rearrange("p (tl f) -> p tl f", f=4)
                l2 = work.tile([128, nt * 2], F32, name="l2")
                nc.vector.tensor_tensor(out=l2[0:_P, :], in0=l13[:, :, 0:2],
                                        in1=l13[:, :, 2:4], op=ALU.add)
                l23 = l2[0:_P, :].rearrange("p (tl f) -> p tl f", f=2)
                s_raw = work.tile([128, nt], F32, name="s_raw")
                nc.vector.tensor_tensor(out=s_raw[0:_P, :], in0=l23[:, :, 0:1],
                                        in1=l23[:, :, 1:2], op=ALU.add)
                obsf = mmp.tile([128, nt], BF16, name="obsf")
                nc.vector.tensor_tensor(out=obsf[0:_P, :], in0=s_raw[0:_P, :],
                                        in1=s_raw[0:_P, :], op=ALU.is_equal)
                s0 = mmp.tile([128, nt], BF16, name="s0")
                nc.vector.scalar_tensor_tensor(out=s0[0:_P, :],
                                               in0=s_raw[0:_P, :],
                                               scalar=3.0e38, in1=obsf[0:_P, :],
                                               op0=ALU.min, op1=ALU.mult)
                return s0, obsf

            def mm_full(s0, obsf, wslot):
                """matmuls + evac + out-DMA for one full 512-col chunk."""
                og = outp.tile([128, _HI * 4 * _C], BF16, name="og")
                # og columns: g*1024 + j*256 + c  with j = 2*jp + jj
                ogv = og[:, :].rearrange("p (g jp jj c) -> p g jp jj c",
                                         g=_HI, jp=2, jj=2)
                for jp in range(2):
                    pp = psp.tile([128, 2048], mybir.dt.float32, name="pp")
                    for jj in range(2):
                        j = 2 * jp + jj
                        lhs_s0 = s0[0:_P, j::4]
                        lhs_ob = obsf[0:_P, j::4]
                        for g in range(_HI):
                            sl = pp[:, (g * 2 + jj) * _C:(g * 2 + jj + 1) * _C]
                            nc.tensor.matmul(sl, lhs_s0, g2m[g][0:_P, 0:_C],
                                             start=True, stop=False)
                            nc.tensor.matmul(sl, lhs_ob, g2m[g][0:_P, _C:2 * _C],
                                             start=False, stop=True)
                    nc.scalar.copy(out=ogv[:, :, jp, :, :],
                                   in_=pp[:].rearrange("p (g jj c) -> p g jj c",
                                                       g=_HI, jj=2))
                nc.scalar.dma_start(out=ov[wslot],
                                    in_=og[:].rearrange("p (g jc) -> p g jc",
                                                        g=_HI))

            def mm_half(s0, obsf, h, wslot):
                """matmuls + evac + out-DMA for one 256-col half chunk."""
                og = outp.tile([128, _HI * 2 * _C], BF16, name="ogh")
                pp = psp.tile([128, 2048], mybir.dt.float32, name="pp")
                for j in range(2):
                    lhs_s0 = s0[0:_P, h * 256 + j:(h + 1) * 256:2]
                    lhs_ob = obsf[0:_P, h * 256 + j:(h + 1) * 256:2]
                    for g in range(_HI):
                        sl = pp[:, (g * 2 + j) * _C:(g * 2 + j + 1) * _C]
                        nc.tensor.matmul(sl, lhs_s0, g2m[g][0:_P, 0:_C],
                                         start=True, stop=False)
                        nc.tensor.matmul(sl, lhs_ob, g2m[g][0:_P, _C:2 * _C],
                                         start=False, stop=True)
                nc.scalar.copy(out=og[:, :], in_=pp[:])
                nc.scalar.dma_start(out=ovw[wslot],
                                    in_=og[:].rearrange("p (g jc) -> p g jc",
                                                        g=_HI))

            for ci in range(_NCHUNK):
                if ci == 0:
                    xt = xt0
                else:
                    xt = xin.tile([128, _SUB, _SW + _PAD], F32, name="xt")
                    with tc.high_priority():
                        load_chunk(ci, xt)
                s0, obsf = fsum(xt)
                mm_full(s0, obsf, ci)
    nc.compile()
    return nc


def _get_nc():
    if "nc" not in _cache:
        _ensure_axon_hooks()
        from concourse import bass_utils
        bass_utils.upload_artifacts = lambda tmpdir: "local://skipped"
        _cache["nc"] = _build()
    return _cache["nc"]


def kernel(input_seq, eq_classes, scale, alpha, edge_present):
    global LAST_RESULTS
    x = np.asarray(input_seq, dtype=np.float32)
    eqc = np.asarray(eq_classes, dtype=np.float32)
    ef = np.asarray(edge_present).astype(np.float32)
    sc = float(np.asarray(scale))
    al = float(np.asarray(alpha))

    import ml_dtypes
    efp = np.zeros((_A, 256), np.float32)
    efp[:, 0:_P] = np.repeat(-sc * ef, _HI, axis=1)
    efp[:, 128:128 + _P] = np.repeat(al * (1.0 - ef), _HI, axis=1)
    tab = np.concatenate([eqc, efp], axis=1).astype(ml_dtypes.bfloat16)
    maskm = np.zeros((128, _HI), np.float32)
    for g in range(_HI):
        maskm[g::_HI, g] = 1.0
    maskm[_P:, :] = 0.0

    nc = _get_nc()
    from concourse import bass_utils
    in_maps = [{"x": np.ascontiguousarray(x[b]), "tab": tab,
                "maskm": maskm} for b in range(_B)]
    trace = bool(os.environ.get("KERNEL_TRACE"))
    res = bass_utils.run_bass_kernel_spmd(nc, in_maps, core_ids=list(range(_B)),
                                          trace=trace)
    LAST_RESULTS = res
    return np.stack([np.asarray(res.results[b]["out"]).astype(np.float32) for b in range(_B)], axis=0)
